# revision 1
# baseline (speedup 1.0000x reference)
"""Causal self-attention kernel for Trainium2, 8 NeuronCores.

Problem: B=2, S=2048, D=1024, H=16 heads, Hd=64. fp32 in/out.
  q/k/v = x @ W{q,k,v}.T + b;  att = softmax(causal(q k^T / 8));  y = att v
  out = y @ Wp.T + bp

Sharding (per spec hint, batch x head-group): core c -> batch b=c//4,
head-group g=c%4 (4 heads = 256 of 1024 dims). Each core computes its
QKV slice on its batch, causal attention for its 4 heads, and a partial
output projection out_c = y_c @ Wp[:, g-slice].T (row-parallel TP).
Host unshard: out[b] = sum_g out_partial[4b+g] + (bv @ Wp.T + bp).

Exact algebraic folds (softmax-invariant / row-sum-1):
  - bk dropped: adds a per-query-row constant to scores -> softmax invariant.
  - bv folded to host: softmax rows sum to 1, so P@(v+bv) = P@v + bv; the bv
    contribution to out is the constant row bv @ Wp.T, added on host.

On-device layout: everything transposed ("S.T layout", [k-part, q-free]) so
causal softmax normalization is per-column, P.T feeds P@V directly as the
moving operand (no PE transposes anywhere), and V carries 64 ones-columns
per head so the matmul emits broadcast softmax denominators for free.

All matmuls run in bf16 (1 PE cycle/row at ANY free size - no fp32r
free-dim>=256 constraint, so no diagonal-block widening), accumulating in
fp32 PSUM. HBM traffic is bf16 end-to-end (x, weights, out), halving DMA
time; the host up/down-casts. Softmax scores stay fp32 in PSUM through
mask-add and exp.

Attention ST blocks are processed in PAIRS sharing one 2-bank PSUM tile so
each exp covers 2 key-blocks: halves the ACT instruction count (each exp
pays ~185ns of SBUF-access overhead), which un-bottlenecks ACT during the
late (long-context) attention chunks. The second diagonal pair packs its
halves at cols (0, 256) so every exp region is contiguous.

Scheduling (all tuned against TimelineSim engine-occupancy traces):
  - QKV/projection work units drip between attention pairs as PE filler at
    the per-pair ACT deficit rate (~170ns/pair); projections are split into
    one-psum-group "minis" (213-427ns) for fine-grained dripping.
  - The last 512 tokens' projection is m-split: the m0 contraction half runs
    as j3 filler, the m1 half + a separate out2 partial (host-summed) is all
    that trails the final head; its normalize is quartered so minis unblock
    progressively, epilogue minis cycle over the freed st/ot/mm psum tags,
    and their copies go 5:3 to ACT (idle after exps) vs DVE (busy with the
    normalize chain).
  - Startup streams x0/weight pieces in consumption order into a 6-way
    interleaved prologue (q00/k00/v0* psum groups open simultaneously);
    wq/wk are packed m-half-major so the m0-only fetch keeps >=512B DMA
    descriptors (smaller runs pay a 2x DMA latency multiplier).
  - QKV-unit copies for t4<=1 go to ACT (idle during j0/j1, while DVE
    queueing would gate the attention stream).
"""
import json
import sys

sys.path.insert(0, "/opt/trn_rl_repo")

import ml_dtypes
import numpy as np

import concourse.bass as bass
import concourse.mybir as mybir
import concourse.tile as tile
from concourse.bass_utils import run_bass_kernel_spmd

F32 = mybir.dt.float32
BF16 = mybir.dt.bfloat16
AF = mybir.ActivationFunctionType
OP = mybir.AluOpType

S = 2048          # tokens per batch (= per core)
D = 1024          # model dim
HL = 4            # heads per core
HD = 64           # head dim
DL = HL * HD      # local dims per core (256)
MASKVAL = -1e30


# ---------------------------------------------------------------------------
# Wait-legalization: the walrus backend enforces <=1 sem-wait per instruction
# (<=2 for EventSemaphore); Tile's wait-assignment can attach more. Spill
# extras onto EventSemaphore instructions inserted before the offender.
def _legalize_waits_json(bir_bytes: bytes) -> bytes:
    j = json.loads(bir_bytes)
    for fn in j["functions"]:
        for bb in fn["blocks"]:
            out = []
            for inst in bb["instructions"]:
                si = inst.get("sync_info") or {}
                ws = si.get("on_wait") or []
                cap = 2 if inst.get("opcode") == "EventSemaphore" else 1
                if len(ws) > cap:
                    extras, keep = ws[:-cap], ws[-cap:]
                    k = 0
                    while extras:
                        chunk, extras = extras[:2], extras[2:]
                        out.append({
                            "debug": inst.get("debug", 0),
                            "engine": inst["engine"],
                            "ins": [],
                            "name": f"{inst['name']}_wfix{k}",
                            "opcode": "EventSemaphore",
                            "outs": [],
                            "sync_info": {"on_update": [], "on_wait": chunk},
                        })
                        k += 1
                    si["on_wait"] = keep
                out.append(inst)
            bb["instructions"] = out
    return json.dumps(j).encode()


def _install_legalizer(nc):
    orig = nc.to_json_bytes
    nc.to_json_bytes = lambda: _legalize_waits_json(orig())


def build_nc() -> bass.Bass:
    nc = bass.Bass(trn_type="TRN2", num_devices=8)

    xT = nc.dram_tensor("xT", [D, S], BF16, kind="ExternalInput")      # x[b].T
    # wq/wk are packed host-side as [p, m2, kc, 128] (m-half-major) so the
    # startup stream can fetch just the m0 half with >=512B descriptors
    wq = nc.dram_tensor("wq", [128, 2048], BF16, kind="ExternalInput")
    wk = nc.dram_tensor("wk", [128, 2048], BF16, kind="ExternalInput")
    wv = nc.dram_tensor("wv", [D, DL], BF16, kind="ExternalInput")     # Wv_g.T
    wp = nc.dram_tensor("wp", [DL, D], BF16, kind="ExternalInput")     # Wp[:,sl].T
    bq = nc.dram_tensor("bq", [DL], F32, kind="ExternalInput")
    mask = nc.dram_tensor("mask", [128, 256], F32, kind="ExternalInput")
    out = nc.dram_tensor("out", [S, D], BF16, kind="ExternalOutput")
    # m1-half partial projection of the last 512 tokens; host adds it to
    # out[1536:2048] (which holds only the m0 half). Splitting the final
    # chunk's projection by contraction halves lets the m0 half run as
    # attention filler and leaves only the m1 half on the drain tail.
    out2 = nc.dram_tensor("out2", [512, D], BF16, kind="ExternalOutput")

    with tile.TileContext(nc) as tc:
        with tc.tile_pool(name="const", bufs=1) as const, \
             tc.tile_pool(name="acts", bufs=1) as acts, \
             tc.tile_pool(name="xin", bufs=2) as xpool, \
             tc.tile_pool(name="pt", bufs=5) as ptp, \
             tc.tile_pool(name="rc", bufs=1) as rcp, \
             tc.tile_pool(name="outp", bufs=4) as outp, \
             tc.tile_pool(name="ps", bufs=1, space="PSUM") as ps:
            wq_sb = const.tile([128, 2, 8, 128], BF16)
            wk_sb = const.tile([128, 2, 8, 128], BF16)
            wv_sb = const.tile([128, 8, DL], BF16)
            wp_sb = const.tile([128, 2, D], BF16)
            bq_sb = const.tile([128, 2], F32)
            # duplicated causal 128x128 triangle (0 / MASKVAL) for one-shot
            # mask adds over both halves of a paired diag ST tile
            mask2_sb = const.tile([128, 2, 128], F32)

            # persistent activations (all bf16)
            qT_sb = [acts.tile([128, S], BF16, name=f"qT{m}") for m in range(2)]
            kT_sb = [acts.tile([128, S], BF16, name=f"kT{m}") for m in range(2)]
            yT_sb = [acts.tile([128, S], BF16, name=f"yT{m}") for m in range(2)]
            # v with interleaved ones-columns: head h at cols [128h,128h+64) = v,
            # [128h+64,128h+128) = 1.0 -> P@V emits broadcast denominators in
            # psum rows 64:128
            vO_sb = [acts.tile([128, 4 * 128], BF16, name=f"vO{i}") for i in range(16)]

            x3 = xT[:].rearrange("(kc p) t -> p kc t", p=128)
            wq4 = wq[:].rearrange("p (m k c) -> p m k c", m=2, k=8)
            wk4 = wk[:].rearrange("p (m k c) -> p m k c", m=2, k=8)
            wv3 = wv[:].rearrange("(kc p) m -> p kc m", p=128)

            xsb_tiles = {}

            def dma_x(t4):
                xsb = xpool.tile([128, 8, 512], BF16, tag="x", name=f"x{t4}")
                xsb_tiles[t4] = xsb
                for half in range(2):
                    nc.sync.dma_start(
                        out=xsb[:, 4 * half:4 * half + 4, :],
                        in_=x3[:, 4 * half:4 * half + 4, 512 * t4:512 * t4 + 512])
                return xsb

            # --- startup stream: interleave weight and x0 pieces in exact
            # consumption order so the first q00 matmuls unblock after two
            # small DMAs and then stream behind the bus ---
            xsb0 = xpool.tile([128, 8, 512], BF16, tag="x", name="x0")
            xsb_tiles[0] = xsb0

            def dma_wqk(wsb, w4, m, half):
                sl = slice(4 * half, 4 * half + 4)
                nc.sync.dma_start(out=wsb[:, m, sl, :], in_=w4[:, m, sl, :])

            def dma_x0(c2):
                sl = slice(2 * c2, 2 * c2 + 2)
                nc.sync.dma_start(out=xsb0[:, sl, :], in_=x3[:, sl, 0:512])

            def dma_wv(c2):
                sl = slice(2 * c2, 2 * c2 + 2)
                nc.sync.dma_start(out=wv_sb[:, sl, :], in_=wv3[:, sl, :])

            # startup stream in consumption order of the interleaved
            # 6-group prologue; m1 weight halves deferred past everything
            # the prologue needs
            dma_wqk(wq_sb, wq4, 0, 0)
            dma_x0(0)
            dma_wqk(wk_sb, wk4, 0, 0)
            dma_wv(0)
            dma_x0(1)
            dma_wv(1)
            dma_x0(2)
            dma_wqk(wq_sb, wq4, 0, 1)
            dma_wqk(wk_sb, wk4, 0, 1)
            dma_wv(2)
            dma_x0(3)
            dma_wv(3)
            nc.sync.dma_start(out=bq_sb, in_=bq[:].rearrange("(m p) -> p m", p=128))
            nc.sync.dma_start(
                out=mask2_sb,
                in_=mask[:].rearrange("p (a c) -> p a c", a=2))
            dma_wqk(wq_sb, wq4, 1, 0)
            dma_wqk(wq_sb, wq4, 1, 1)
            dma_wqk(wk_sb, wk4, 1, 0)
            dma_wqk(wk_sb, wk4, 1, 1)

            def dma_wp():
                # deferred past the x1 prefetch: wp is first consumed at
                # j1-h3 (~30us), x1 at j0-h1 (~17us)
                for m in range(2):
                    nc.sync.dma_start(
                        out=wp_sb[:, m, :],
                        in_=wp[:].rearrange("(m p) n -> p m n", p=128)[:, m, :])
            for i in range(16):
                v3 = vO_sb[i].rearrange("p (h c) -> p h c", h=HL)
                nc.gpsimd.memset(v3[:, :, HD:128], 1.0)

            # --- keyed work units (one PE psum-group each), interleaved into
            # the attention stream as filler so the in-order PE never starves
            # while ACT chews exps ---
            def unit_qk(which, t4, m):
                wsb, dst = (wq_sb, qT_sb) if which == "q" else (wk_sb, kT_sb)

                def go():
                    ts512 = slice(512 * t4, 512 * t4 + 512)
                    xsb = xsb_tiles[t4]
                    p = ps.tile([128, 512], F32, tag="mm", bufs=2,
                                name=f"{which}{t4}{m}")
                    for kc in range(8):
                        nc.tensor.matmul(p[:, :], wsb[:, m, kc, :], xsb[:, kc, :],
                                         start=(kc == 0), stop=(kc == 7))
                    # t4<=1 units run during j0/j1 where ACT idles and DVE
                    # queueing gates the attention stream - copy there
                    if which == "q":
                        if t4 <= 1:
                            nc.scalar.add(dst[m][:, ts512], p, bq_sb[:, m:m + 1])
                        else:
                            nc.vector.tensor_scalar_add(dst[m][:, ts512], p,
                                                        bq_sb[:, m:m + 1])
                    elif t4 <= 1:
                        nc.scalar.copy(dst[m][:, ts512], p)
                    else:
                        nc.vector.tensor_copy(dst[m][:, ts512], p)
                return go

            def unit_v(t4, si):
                def go():
                    xsb = xsb_tiles[t4]
                    tl = slice(128 * si, 128 * si + 128)
                    p = ps.tile([128, DL], F32, tag="mm", bufs=2, name=f"v{t4}{si}")
                    for kc in range(8):
                        nc.tensor.matmul(p[:, :], xsb[:, kc, tl], wv_sb[:, kc, :],
                                         start=(kc == 0), stop=(kc == 7))
                    v3 = vO_sb[4 * t4 + si].rearrange("p (h c) -> p h c", h=HL)
                    if t4 <= 1:
                        nc.scalar.copy(
                            v3[:, :, 0:HD],
                            p[:].rearrange("p (h c) -> p h c", h=HL))
                    else:
                        nc.vector.tensor_copy(
                            v3[:, :, 0:HD],
                            p[:].rearrange("p (h c) -> p h c", h=HL))
                return go

            def pj_minis(jj, sp, m2s=(0, 1), fine=False, tags=None, cps=None):
                """Projection of the 256 tokens at 512*jj+256*sp over the
                contraction halves in m2s, as FOUR independent mini-units
                (one psum group, 213-427ns of PE each) so they can drip
                between attention pairs at the per-pair ACT deficit rate.
                A single-half m2s=(1,) unit writes its partial to out2[]
                (host sums); m2s=(0,) writes out[] (sole contribution)."""
                t0 = 512 * jj + 256 * sp
                dram = out2 if m2s == (1,) else out
                r0 = t0 - 1536 if m2s == (1,) else t0
                state = {}

                def mini(sub, ncol, tag, cp):
                    def go():
                        if "osb" not in state:
                            state["osb"] = outp.tile(
                                [128, 2, D], BF16, tag="o",
                                name=f"o{jj}{sp}{m2s[0]}{len(m2s)}")
                        osb = state["osb"]
                        shape = [128, 1024] if tag == "st" else [128, 512]
                        pj = ps.tile(shape, F32, tag=tag, bufs=2,
                                     name=f"pj{jj}{sp}{sub}{ncol}{m2s[0]}")
                        for ii, m2 in enumerate(m2s):
                            nc.tensor.matmul(
                                pj[:, 0:512],
                                yT_sb[m2][:, t0 + 128 * sub:t0 + 128 * sub + 128],
                                wp_sb[:, m2, 512 * ncol:512 * ncol + 512],
                                start=(ii == 0), stop=(ii == len(m2s) - 1))
                        dst = osb[:, sub, 512 * ncol:512 * ncol + 512]
                        if cp == "a":
                            nc.scalar.copy(dst, pj[:, 0:512])
                        else:
                            nc.vector.tensor_copy(dst, pj[:, 0:512])
                        if fine and sp == 1 and sub == 1:
                            # very last sub: per-512-col DMAs so the final
                            # transfer is small (364ns); the LAST one on SP
                            # (DGE delay 650 vs scalar's 784)
                            q = nc.sync if ncol == 1 else nc.scalar
                            q.dma_start(
                                out=dram[r0 + 128:r0 + 256,
                                         512 * ncol:512 * ncol + 512],
                                in_=dst)
                        elif fine and ncol == 1:
                            # one [128,1024] DMA per 128-token sub: fewer
                            # HWDGE slots (625ns each, globally serialized)
                            # on the drain tail
                            nc.sync.dma_start(
                                out=dram[r0 + 128 * sub:r0 + 128 * sub + 128, :],
                                in_=osb[:, sub, :])
                        elif not fine and sub == 1 and ncol == 1:
                            # one DMA for the whole 256-token piece
                            nc.sync.dma_start(
                                out=dram[r0:r0 + 256, :].rearrange(
                                    "(a p) n -> p a n", p=128),
                                in_=osb[:])
                    return go

                tags = tags or ["mm"] * 4
                # default copy-engine split: DVE for ncol0, ACT for ncol1 in
                # fine mode; DVE otherwise
                cps = cps or [("a" if (fine and ncol == 1) else "v")
                              for _ in range(2) for ncol in range(2)]
                return [mini(sub, ncol, tags[2 * sub + ncol],
                             cps[2 * sub + ncol])
                        for sub in range(2) for ncol in range(2)]

            # prologue: q00/k00/v0* psum groups all open simultaneously,
            # emitted per-kc so each arriving x0/weight piece unlocks ~850ns
            # of PE work (the startup stream is DMA-bound otherwise). Tags
            # borrowed so every tag's first allocation has its full size:
            # mm <- q00,k00 [128,512]; ot <- v0,v1 (written [0:256)); st <-
            # v2,v3 ([128,1024] slots, written [0:256)).
            pq = ps.tile([128, 512], F32, tag="mm", bufs=2, name="pq00")
            pk = ps.tile([128, 512], F32, tag="mm", bufs=2, name="pk00")
            pv = [ps.tile([128, 512], F32, tag="ot", bufs=2, name=f"pv{si}")
                  for si in range(2)]
            pv += [ps.tile([128, 1024], F32, tag="st", bufs=2, name=f"pv{si}")
                   for si in range(2, 4)]
            for kc in range(8):
                nc.tensor.matmul(pq[:, :], wq_sb[:, 0, kc, :], xsb0[:, kc, :],
                                 start=(kc == 0), stop=(kc == 7))
                nc.tensor.matmul(pk[:, :], wk_sb[:, 0, kc, :], xsb0[:, kc, :],
                                 start=(kc == 0), stop=(kc == 7))
                for si in range(4):
                    nc.tensor.matmul(
                        pv[si][:, 0:DL],
                        xsb0[:, kc, 128 * si:128 * si + 128], wv_sb[:, kc, :],
                        start=(kc == 0), stop=(kc == 7))
            nc.vector.tensor_scalar_add(qT_sb[0][:, 0:512], pq, bq_sb[:, 0:1])
            nc.vector.tensor_copy(kT_sb[0][:, 0:512], pk)
            for si in range(4):
                v3 = vO_sb[si].rearrange("p (h c) -> p h c", h=HL)
                nc.vector.tensor_copy(
                    v3[:, :, 0:HD],
                    pv[si][:, 0:DL].rearrange("p (h c) -> p h c", h=HL))

            for j in range(4):
                if j + 1 < 4:
                    dma_x(j + 1)
                if j == 0:
                    dma_wp()
                ni = 4 * (j + 1)
                npairs = ni // 2
                # per-head filler queues (emission-order deadlines):
                #  h0: v(j,*)  (v[4j+si] consumed by h0's diag OTs)
                #  h1: q/k(j,m1)  (consumed from h2)
                #  h2/h3: q/k(j+1,m0) (next j's h0) and proj(j-1) (no deadline)
                if j == 3:
                    # the m0 half of the final chunk's projection is legal
                    # filler once h1's normalize lands; its mini-groups drip
                    # between pairs to cover the per-pair ACT deficit
                    fills = {
                        0: [unit_v(3, si) for si in range(4)],
                        1: [unit_qk("q", 3, 1), unit_qk("k", 3, 1)],
                        2: pj_minis(2, 0) + pj_minis(2, 1),
                        3: pj_minis(3, 0, m2s=(0,)) + pj_minis(3, 1, m2s=(0,)),
                    }
                elif j > 0:
                    fills = {
                        0: [unit_v(j, si) for si in range(4)],
                        1: [unit_qk("q", j, 1), unit_qk("k", j, 1)],
                        2: [unit_qk("q", j + 1, 0), unit_qk("k", j + 1, 0)],
                        3: pj_minis(j - 1, 0) + pj_minis(j - 1, 1),
                    }
                else:
                    # m1 QKV at h0 so qT[1]/kT[1] land well before h2's
                    # first ST needs them (their weights arrive ~9us, h0
                    # runs ~13us)
                    fills = {
                        0: [unit_qk("q", 0, 1), unit_qk("k", 0, 1)],
                        1: [],
                        2: [unit_qk("q", 1, 0)],
                        3: [unit_qk("k", 1, 0)],
                    }
                for h in range(HL):
                    m, po = h // 2, 64 * (h % 2)
                    qh = qT_sb[m][po:po + 64, :]
                    kh = kT_sb[m][po:po + 64, :]
                    ot = ps.tile([128, 512], F32, tag="ot", bufs=2, name=f"ot{j}{h}")
                    fill = fills[h]
                    # pop positions: spread evenly so filler dribbles at the
                    # per-pair ACT deficit rate; h0's v units clamp to the
                    # pair before the diagonal (vO[4j..] consumption deadline)
                    pops = {}
                    for k, fn in enumerate(fill):
                        pos = (k * npairs) // len(fill)
                        if j == 3 and h >= 2:
                            # exp-backlog drift peaks late in these heads;
                            # the first two pairs ride on LOOKP slack alone
                            pos += 2
                        if h == 0 and j > 0:
                            pos = min(pos, 2 * j - 1)
                        pops.setdefault(min(pos, npairs - 1), []).append(fn)

                    # One pair = 2 key-blocks sharing a 2-bank psum tile and a
                    # single exp. Half 1 packs right after half 0's written
                    # extent (rounded to 256) so the exp region is always
                    # contiguous with no dead columns.
                    def emit_pair_st(pp):
                        st = ps.tile([128, 1024], F32, tag="st", bufs=2,
                                     name=f"st{j}{h}{pp}")
                        pt = ptp.tile([128, 1024], BF16, tag="pt",
                                      name=f"pt{j}{h}{pp}")
                        i0 = 2 * pp
                        qs0 = max(512 * j, 128 * i0)
                        n0 = 512 * j + 512 - qs0
                        off1 = max(256, n0)
                        infos = []
                        for half in range(2):
                            i = i0 + half
                            qs = max(512 * j, 128 * i)
                            n = 512 * j + 512 - qs
                            o = off1 * half
                            nc.tensor.matmul(
                                st[:, o:o + n],
                                kh[:, 128 * i:128 * i + 128], qh[:, qs:qs + n],
                                start=True, stop=True)
                            infos.append((i, qs, n, o))
                        if i0 >= 4 * j:  # diagonal pair: causal triangles
                            stv = st[:, 0:2 * off1].rearrange(
                                "p (a c) -> p a c", a=2)
                            nc.vector.tensor_tensor(
                                stv[:, :, 0:128], stv[:, :, 0:128], mask2_sb,
                                op=OP.add)
                        nc.scalar.activation(pt[:, 0:off1 + infos[1][2]],
                                             st[:, 0:off1 + infos[1][2]],
                                             AF.Exp, scale=0.125)
                        return pt, infos

                    def emit_ot_pair(pt, infos):
                        for i, qs, n, o in infos:
                            nc.tensor.matmul(
                                ot[:, qs - 512 * j:512],
                                vO_sb[i][:, 128 * h:128 * h + 128],
                                pt[:, o:o + n],
                                start=(i == 0), stop=(i == ni - 1))

                    LOOKP = 2
                    pairs = {}
                    for pp in range(min(LOOKP, npairs)):
                        pairs[pp] = emit_pair_st(pp)
                    for pp in range(npairs):
                        if pp + LOOKP < npairs:
                            pairs[pp + LOOKP] = emit_pair_st(pp + LOOKP)
                        for fn in pops.pop(pp, []):
                            fn()
                        emit_ot_pair(*pairs.pop(pp))
                    for fns in [pops.pop(k) for k in sorted(pops)]:
                        for fn in fns:
                            fn()
                    # normalize: ot rows 64:128 hold broadcast denominators;
                    # reciprocal on DVE (slack) - ACT stays on exps
                    rb = rcp.tile([64, 512], F32, tag=f"rb{h}", name=f"rb{j}{h}")
                    ydst = yT_sb[m][po:po + 64, 512 * j:512 * j + 512]
                    if j == 3 and h == 3:
                        # quarter the recip+mult chain: epilogue mini (sub)
                        # k needs only ydst quarter k, so the first one
                        # unblocks after ~550ns instead of ~1.3us
                        for qt in range(4):
                            cs = slice(128 * qt, 128 * qt + 128)
                            nc.vector.reciprocal(rb[:, cs], ot[64:128, cs])
                            nc.vector.tensor_tensor(
                                ydst[:, cs], ot[0:64, cs], rb[:, cs],
                                op=OP.mult)
                    else:
                        nc.vector.reciprocal(rb, ot[64:128, :])
                        nc.vector.tensor_tensor(ydst, ot[0:64, :], rb,
                                                op=OP.mult)
            # epilogue minis cycle across the now-free st/ot psum tags (6
            # slots in flight) so their psum->sbuf copies stream on DVE+ACT
            # instead of throttling the matmuls through 2 mm slots. Order by
            # slot availability: mm and h2's ot slot free first; st slots
            # free only once h3's final exps drain; h3's own ot frees after
            # the normalize reads.
            # copies: ACT takes the first five (it idles between exps by
            # now); DVE takes the last three AFTER its normalize chain, so
            # the norm quarters are never queued behind 658ns copies
            tagcycle = ["st", "ot", "mm", "st", "ot", "mm", "st", "ot"]
            cpcycle = ["a", "a", "a", "a", "a", "v", "v", "v"]
            for sp in range(2):
                for mini in pj_minis(3, sp, m2s=(1,), fine=True,
                                     tags=tagcycle[4 * sp:4 * sp + 4],
                                     cps=cpcycle[4 * sp:4 * sp + 4]):
                    mini()

    _install_legalizer(nc)
    return nc


_NC_CACHE = None


def _get_nc():
    global _NC_CACHE
    if _NC_CACHE is None:
        _NC_CACHE = build_nc()
    return _NC_CACHE


def _bf16(a: np.ndarray) -> np.ndarray:
    return np.ascontiguousarray(np.asarray(a, np.float32)).astype(ml_dtypes.bfloat16)


def _pack_mmajor(wT: np.ndarray) -> np.ndarray:
    """[D, DL] (d = kc*128+p, col = m2*128+mi) -> [p, (m2, kc, mi)] flat."""
    a = np.asarray(wT, np.float32).reshape(8, 128, 2, 128)
    return _bf16(a.transpose(1, 2, 0, 3).reshape(128, 2048))


def make_in_maps(x, Wq, bq, Wk, Wv, Wp):
    x = np.asarray(x, np.float32)
    xT = [_bf16(x[b].T) for b in range(2)]
    tri = np.where(np.arange(128)[None, :] >= np.arange(128)[:, None],
                   np.float32(0.0), np.float32(MASKVAL)).astype(np.float32)
    tri2 = np.concatenate([tri, tri], axis=1)  # [128, 256] duplicated
    in_maps = []
    for c in range(8):
        b, g = c // 4, c % 4
        sl = slice(DL * g, DL * g + DL)
        in_maps.append({
            "xT": xT[b],
            "wq": _pack_mmajor(np.asarray(Wq)[sl, :].T),
            "wk": _pack_mmajor(np.asarray(Wk)[sl, :].T),
            "wv": _bf16(np.asarray(Wv)[sl, :].T),
            "wp": _bf16(np.asarray(Wp)[:, sl].T),
            "bq": np.ascontiguousarray(np.asarray(bq, np.float32)[sl]),
            "mask": tri2,
        })
    return in_maps


def kernel(x, Wq, bq, Wk, bk, Wv, bv, Wp, bp, _run_kwargs=None):
    nc = _get_nc()
    in_maps = make_in_maps(x, Wq, bq, Wk, Wv, Wp)
    res = run_bass_kernel_spmd(nc, in_maps, list(range(8)), **(_run_kwargs or {}))
    corr = (np.asarray(bv, np.float32) @ np.asarray(Wp, np.float32).T
            + np.asarray(bp, np.float32))
    out = np.zeros((2, S, D), np.float32)
    for c in range(8):
        out[c // 4] += np.asarray(res.results[c]["out"]).astype(np.float32)
        out[c // 4, 1536:2048] += np.asarray(
            res.results[c]["out2"]).astype(np.float32)
    out += corr[None, None, :]
    kernel.last_results = res
    return out



# revision 40
# speedup vs baseline: 1.1949x; 1.1949x over previous
"""Causal self-attention kernel for Trainium2, 8 NeuronCores (fp8 v2).

Problem: B=2, S=2048, D=1024, H=16 heads, Hd=64. fp32 in/out.
  q/k/v = x @ W{q,k,v}.T + b;  att = softmax(causal(q k^T / 8));  y = att v
  out = y @ Wp.T + bp

Sharding (batch x head-group): core c -> batch b=c//4, head-group g=c%4
(4 heads = 256 of 1024 dims). Each core computes its QKV slice on its
batch, causal attention for its 4 heads, and a partial output projection
out_c = y_c @ Wp[:, g-slice].T (row-parallel TP). Host unshard:
out[b] = sum_g out_partial[4b+g] + (bv @ Wp.T + bp).

fp8 acceleration (validated vs the 2e-2 gate; numpy study ~1.4e-2):
  - q/k PRODUCTION runs in fp8e4m3 DoubleRow (x8 @ 32*Wq8): both operands
    fp8, 2 contraction rows/PE-cycle => half the bf16 matmul time. Weights
    scaled x32 into e4m3 normal range (x32 is exponent-exact in fp8/psum);
    bias 32*bq folded in the psum->sbuf copy; exp scale becomes 0.125/1024.
  - SCORES (q k^T) also run DoubleRow: qT/kT live as fp8 [64, 2, S] tiles
    (heads 0-1 / 2-3 per tile, head-dim halves side by side in the free
    dim). The psum->sbuf q/k copies write halves straight (psum[0:64]) and
    partition-REMAPPED (psum[64:128] -> partitions 0-63; engines accept
    differing in/out base partitions - validated on HW).
  - v/OT/projection stay bf16: quantizing v, P, or y to fp8 measured over
    the 2e-2 budget (2.5-3e-2), so OT cost is unchanged.
PE row budget: QKV 65536 + ST 34816 + OT 69632 + proj 32768 = 202752 rows
(~84.5us at 2.4GHz) vs 270336 bf16 rows (~112.6us) in the baseline.

Exact algebraic folds (softmax-invariant / row-sum-1):
  - bk dropped: per-query-row constant -> softmax invariant.
  - bv folded to host: softmax rows sum to 1 -> bv @ Wp.T added on host.

On-device layout: transposed "S.T layout" ([k-part, q-free]) so causal
softmax normalization is per-column, P.T feeds P@V directly as the moving
operand, and V carries 64 ones-columns per head so the matmul emits
broadcast softmax denominators for free. Attention ST blocks run in PAIRS
sharing a 2-bank PSUM tile so each exp covers 2 key-blocks (halves ACT
instruction overhead, ~185ns each).

Scheduling: QKV/projection work drips between attention pairs as PE
filler (per-pair ACT deficit ~0.4us with fp8 ST). Chunk j+1's q/k units
MUST finish inside chunk j (every head now needs both head-dim slots
before its first ST). Engine split: prologue-era copies on ACT (idle
pre-attention), all later copies/normalize/mask on DVE; exps own ACT.
Startup streams x8 (q/k path) before bf16 x (v path, token-split pieces
so v00/v01 meet the first diagonal OT); j3 hosts proj(1..3) minis since
late chunks have the largest exp-vs-PE deficit.
"""
import json
import sys

sys.path.insert(0, "/opt/trn_rl_repo")

import ml_dtypes
import numpy as np

import concourse.bass as bass
import concourse.mybir as mybir
import concourse.tile as tile
from concourse.bass_utils import run_bass_kernel_spmd

F32 = mybir.dt.float32
BF16 = mybir.dt.bfloat16
E4 = mybir.dt.float8e4
AF = mybir.ActivationFunctionType
OP = mybir.AluOpType
DR = mybir.MatmulPerfMode.DoubleRow

S = 2048          # tokens per batch (= per core)
D = 1024          # model dim
HL = 4            # heads per core
HD = 64           # head dim
DL = HL * HD      # local dims per core (256)
MASKVAL = -1e30
ESCALE = 0.125 / 1024.0   # exp scale: 1/sqrt(Hd) / (32*32 weight prescale)


# ---------------------------------------------------------------------------
# Wait-legalization: the walrus backend enforces <=1 sem-wait per instruction
# (<=2 for EventSemaphore); Tile's wait-assignment can attach more. Spill
# extras onto EventSemaphore instructions inserted before the offender.
def _legalize_waits_json(bir_bytes: bytes) -> bytes:
    j = json.loads(bir_bytes)
    for fn in j["functions"]:
        for bb in fn["blocks"]:
            out = []
            for inst in bb["instructions"]:
                si = inst.get("sync_info") or {}
                ws = si.get("on_wait") or []
                cap = 2 if inst.get("opcode") == "EventSemaphore" else 1
                if len(ws) > cap:
                    extras, keep = ws[:-cap], ws[-cap:]
                    k = 0
                    while extras:
                        chunk, extras = extras[:2], extras[2:]
                        out.append({
                            "debug": inst.get("debug", 0),
                            "engine": inst["engine"],
                            "ins": [],
                            "name": f"{inst['name']}_wfix{k}",
                            "opcode": "EventSemaphore",
                            "outs": [],
                            "sync_info": {"on_update": [], "on_wait": chunk},
                        })
                        k += 1
                    si["on_wait"] = keep
                out.append(inst)
            bb["instructions"] = out
    return json.dumps(j).encode()


def _install_legalizer(nc):
    orig = nc.to_json_bytes
    nc.to_json_bytes = lambda: _legalize_waits_json(orig())


def build_nc() -> bass.Bass:
    nc = bass.Bass(trn_type="TRN2", num_devices=8)

    xT = nc.dram_tensor("xT", [D, S], BF16, kind="ExternalInput")      # x[b].T
    # fp8 x for the q/k path, 64-partition layout: [p6, (kc, s, t)] with
    # model-dim d = 128*kc + 64*s + p6 (DoubleRow pair = (p6, s)). All 8
    # kc chunks at base partition 0: the PE rejects accumulation groups
    # whose operands switch base partitions mid-group.
    x8 = nc.dram_tensor("x8", [64, 8 * 2 * S], E4, kind="ExternalInput")
    # fp8 scaled weights: [p6, (s_out, kc, s_in, pi)]; s_out-major so the
    # startup stream fetches one out-slot half with 2KB descriptors
    wq8 = nc.dram_tensor("wq8", [64, 4096], E4, kind="ExternalInput")
    wk8 = nc.dram_tensor("wk8", [64, 4096], E4, kind="ExternalInput")
    wv = nc.dram_tensor("wv", [D, DL], BF16, kind="ExternalInput")     # Wv_g.T
    wp = nc.dram_tensor("wp", [DL, D], BF16, kind="ExternalInput")     # Wp[:,sl].T
    bq = nc.dram_tensor("bq", [128, 2], F32, kind="ExternalInput")     # 32*bq perm
    # 0/1 lower-triangle (duplicated x2): multiplied into pt AFTER the exp
    # on the Pool engine (bf16 SBUF-only op), keeping the causal mask off
    # the DVE/ACT critical path entirely
    mask = nc.dram_tensor("mask", [128, 256], BF16, kind="ExternalInput")
    out = nc.dram_tensor("out", [S, D], BF16, kind="ExternalOutput")
    # m1-half partial projection of the last 512 tokens; host adds it to
    # out[1536:2048] (which holds only the m0 half).
    out2 = nc.dram_tensor("out2", [512, D], BF16, kind="ExternalOutput")

    with tile.TileContext(nc) as tc:
        with tc.tile_pool(name="const", bufs=1) as const, \
             tc.tile_pool(name="acts", bufs=1) as acts, \
             tc.tile_pool(name="xin", bufs=2) as xpool, \
             tc.tile_pool(name="pt", bufs=5) as ptp, \
             tc.tile_pool(name="rc", bufs=1) as rcp, \
             tc.tile_pool(name="outp", bufs=4) as outp, \
             tc.tile_pool(name="ps", bufs=1, space="PSUM") as ps:
            wq8_sb = const.tile([64, 2, 8, 2, 128], E4)
            wk8_sb = const.tile([64, 2, 8, 2, 128], E4)
            wv_sb = const.tile([128, 8, DL], BF16)
            wp_sb = const.tile([128, 2, D], BF16)
            bq_sb = const.tile([128, 2], F32)
            mask2_sb = const.tile([128, 2, 128], BF16)

            # q/k in fp8 "DoubleRow ST" layout: tile a = heads 0,1 / b =
            # heads 2,3; head h at partitions 32*(h%2)..+32, head-dim
            # d = 32*slot + partition-offset
            qTa = acts.tile([64, 2, S], E4, name="qTa")
            qTb = acts.tile([64, 2, S], E4, name="qTb")
            kTa = acts.tile([64, 2, S], E4, name="kTa")
            kTb = acts.tile([64, 2, S], E4, name="kTb")
            yT_sb = [acts.tile([128, S], BF16, name=f"yT{m}") for m in range(2)]
            # v with interleaved ones-columns: head h at cols [128h,128h+64) = v,
            # [128h+64,128h+128) = 1.0 -> P@V emits broadcast denominators
            vO_sb = [acts.tile([128, 4 * 128], BF16, name=f"vO{i}") for i in range(16)]

            x3 = xT[:].rearrange("(kc p) t -> p kc t", p=128)
            x83 = x8[:].rearrange("p (kc s t) -> p kc s t", kc=8, s=2)
            wq84 = wq8[:].rearrange("p (so kc si x) -> p so kc si x",
                                    so=2, kc=8, si=2)
            wk84 = wk8[:].rearrange("p (so kc si x) -> p so kc si x",
                                    so=2, kc=8, si=2)
            wv3 = wv[:].rearrange("(kc p) m -> p kc m", p=128)

            xsb_tiles = {}
            x8sb_tiles = {}

            def dma_x8(t4):
                # prefetched TWO chunks ahead: chunk t4's q/k fill units
                # drip during chunk t4-1 and block the in-order PE if their
                # x8 hasn't landed
                x8sb = xpool.tile([64, 8, 2, 512], E4, tag="x8", bufs=3,
                                  name=f"x8{t4}")
                x8sb_tiles[t4] = x8sb
                for half in range(2):
                    nc.sync.dma_start(
                        out=x8sb[:, 4 * half:4 * half + 4],
                        in_=x83[:, 4 * half:4 * half + 4, :,
                                512 * t4:512 * t4 + 512])

            def dma_xbf(t4):
                xsb = xpool.tile([128, 8, 512], BF16, tag="x", name=f"x{t4}")
                xsb_tiles[t4] = xsb
                for half in range(2):
                    nc.sync.dma_start(
                        out=xsb[:, 4 * half:4 * half + 4, :],
                        in_=x3[:, 4 * half:4 * half + 4, 512 * t4:512 * t4 + 512])

            # --- startup stream in prologue-consumption order ---
            x8sb0 = xpool.tile([64, 8, 2, 512], E4, tag="x8", bufs=3,
                               name="x80")
            x8sb_tiles[0] = x8sb0
            xsb0 = xpool.tile([128, 8, 512], BF16, tag="x", name="x0")
            xsb_tiles[0] = xsb0

            def dma_x0tok(tp, n):
                # token-split pieces: v00/v01 (tokens 0-255) unblock early
                sl = slice(128 * tp, 128 * tp + n)
                nc.sync.dma_start(out=xsb0[:, :, sl], in_=x3[:, :, sl])

            # x8 pieces go through the Pool SWDGE queue: desc-gen on the
            # idle Pool engine runs PARALLEL to the HWDGE's ~650ns/DMA
            # serialization, so the weight stream isn't pushed back
            nc.gpsimd.dma_start(out=x8sb0[:, 0:4], in_=x83[:, 0:4, :, 0:512])
            nc.gpsimd.dma_start(out=x8sb0[:, 4:8], in_=x83[:, 4:8, :, 0:512])
            nc.sync.dma_start(out=wq8_sb, in_=wq84)
            nc.sync.dma_start(out=bq_sb, in_=bq[:])
            nc.sync.dma_start(out=wk8_sb, in_=wk84)
            nc.sync.dma_start(out=wv_sb, in_=wv3)
            dma_x0tok(0, 256)
            nc.sync.dma_start(
                out=mask2_sb,
                in_=mask[:].rearrange("p (a c) -> p a c", a=2))
            dma_x0tok(2, 256)
            dma_x8(1)

            def dma_wp():
                # deferred: wp first consumed by proj(0) minis at j2-h3
                for m in range(2):
                    nc.sync.dma_start(
                        out=wp_sb[:, m, :],
                        in_=wp[:].rearrange("(m p) n -> p m n", p=128)[:, m, :])
            for i in range(16):
                v3 = vO_sb[i].rearrange("p (h c) -> p h c", h=HL)
                nc.gpsimd.memset(v3[:, :, HD:128], 1.0)

            # --- keyed work units (one PE psum-group each) ---
            def unit_qk(which, t4, s):
                wsb = wq8_sb if which == "q" else wk8_sb
                da, db = (qTa, qTb) if which == "q" else (kTa, kTb)

                def go():
                    ts512 = slice(512 * t4, 512 * t4 + 512)
                    x8sb = x8sb_tiles[t4]
                    p = ps.tile([128, 512], F32, tag="mm", bufs=2,
                                name=f"{which}{t4}{s}")
                    for kc in range(8):
                        nc.tensor.matmul(
                            p[:, :], wsb[:, s, kc, :, :], x8sb[:, kc, :, :],
                            start=(kc == 0), stop=(kc == 7), perf_mode=DR)
                    # psum[0:64] -> heads 0,1 straight; [64:128] -> heads 2,3
                    # partition-remapped into the b tile
                    if which == "q":
                        nc.vector.tensor_scalar_add(
                            da[:, s, ts512], p[0:64, :], bq_sb[0:64, s:s + 1])
                        nc.vector.tensor_scalar_add(
                            db[:, s, ts512], p[64:128, :],
                            bq_sb[64:128, s:s + 1])
                    else:
                        nc.vector.tensor_copy(da[:, s, ts512], p[0:64, :])
                        nc.vector.tensor_copy(db[:, s, ts512], p[64:128, :])
                return go

            def unit_v(t4, si, tag="mm"):
                def go():
                    xsb = xsb_tiles[t4]
                    tl = slice(128 * si, 128 * si + 128)
                    shape = [128, 1024] if tag == "st" else [128, DL]
                    p = ps.tile(shape, F32, tag=tag, bufs=2, name=f"v{t4}{si}")
                    for kc in range(8):
                        nc.tensor.matmul(p[:, 0:DL], xsb[:, kc, tl],
                                         wv_sb[:, kc, :],
                                         start=(kc == 0), stop=(kc == 7))
                    v3 = vO_sb[4 * t4 + si].rearrange("p (h c) -> p h c", h=HL)
                    nc.vector.tensor_copy(
                        v3[:, :, 0:HD],
                        p[:, 0:DL].rearrange("p (h c) -> p h c", h=HL))
                return go

            def pj_minis(jj, sp, m2s=(0, 1), fine=False, tags=None, cps=None):
                """Projection of the 256 tokens at 512*jj+256*sp over the
                contraction halves in m2s, as FOUR independent mini-units
                (one psum group each) dripped between attention pairs."""
                t0 = 512 * jj + 256 * sp
                dram = out2 if m2s == (1,) else out
                r0 = t0 - 1536 if m2s == (1,) else t0
                state = {}

                def mini(sub, ncol, tag, cp):
                    def go():
                        if "osb" not in state:
                            state["osb"] = outp.tile(
                                [128, 2, D], BF16, tag="o",
                                name=f"o{jj}{sp}{m2s[0]}{len(m2s)}")
                        osb = state["osb"]
                        shape = [128, 1024] if tag == "st" else [128, 512]
                        pj = ps.tile(shape, F32, tag=tag, bufs=2,
                                     name=f"pj{jj}{sp}{sub}{ncol}{m2s[0]}")
                        for ii, m2 in enumerate(m2s):
                            nc.tensor.matmul(
                                pj[:, 0:512],
                                yT_sb[m2][:, t0 + 128 * sub:t0 + 128 * sub + 128],
                                wp_sb[:, m2, 512 * ncol:512 * ncol + 512],
                                start=(ii == 0), stop=(ii == len(m2s) - 1))
                        dst = osb[:, sub, 512 * ncol:512 * ncol + 512]
                        if cp == "a":
                            nc.scalar.copy(dst, pj[:, 0:512])
                        else:
                            nc.vector.tensor_copy(dst, pj[:, 0:512])
                        if fine and sp == 1 and sub == 1:
                            q = nc.sync if ncol == 1 else nc.scalar
                            q.dma_start(
                                out=dram[r0 + 128:r0 + 256,
                                         512 * ncol:512 * ncol + 512],
                                in_=dst)
                        elif fine and ncol == 1:
                            nc.sync.dma_start(
                                out=dram[r0 + 128 * sub:r0 + 128 * sub + 128, :],
                                in_=osb[:, sub, :])
                        elif not fine and sub == 1 and ncol == 1:
                            nc.sync.dma_start(
                                out=dram[r0:r0 + 256, :].rearrange(
                                    "(a p) n -> p a n", p=128),
                                in_=osb[:])
                    return go

                tags = tags or ["mm"] * 4
                cps = cps or [("a" if (fine and ncol == 1) else "v")
                              for _ in range(2) for ncol in range(2)]
                return [mini(sub, ncol, tags[2 * sub + ncol],
                             cps[2 * sub + ncol])
                        for sub in range(2) for ncol in range(2)]

            # --- prologue: q00 s0/s1 (mm), k00 s0/s1 (ot), v00/v01
            # (st-borrow), each group emitted WHOLE in DMA-arrival order so
            # completions stagger; the head-0/1 "a" copies go first so the
            # first ST unblocks after 4 copies (b copies feed h2, v feeds
            # the first diagonal OT - both later deadlines)
            # psum borrows: q00 -> mm, k00 -> ot, and v00/v01 ALSO -> mm
            # (NOT st: the first attention pair's STs must find fresh st
            # slots, not wait on the v-copy chain behind the big x0/wv DMAs)
            pq = [ps.tile([128, 512], F32, tag="mm", bufs=2, name=f"pq00{s}")
                  for s in range(2)]
            pk = [ps.tile([128, 512], F32, tag="ot", bufs=2, name=f"pk00{s}")
                  for s in range(2)]

            def mm_qk(p, wsb, s):
                for kc in range(8):
                    nc.tensor.matmul(
                        p[:, :], wsb[:, s, kc, :, :], x8sb0[:, kc, :, :],
                        start=(kc == 0), stop=(kc == 7), perf_mode=DR)

            mm_qk(pq[0], wq8_sb, 0)
            mm_qk(pk[0], wk8_sb, 0)
            nc.vector.tensor_scalar_add(qTa[:, 0, 0:512], pq[0][0:64, :],
                                        bq_sb[0:64, 0:1])
            nc.vector.tensor_copy(kTa[:, 0, 0:512], pk[0][0:64, :])
            mm_qk(pq[1], wq8_sb, 1)
            mm_qk(pk[1], wk8_sb, 1)
            nc.vector.tensor_scalar_add(qTa[:, 1, 0:512], pq[1][0:64, :],
                                        bq_sb[0:64, 1:2])
            nc.vector.tensor_copy(kTa[:, 1, 0:512], pk[1][0:64, :])
            # b copies (heads 2,3) on ACT: idle until the first exp, and
            # keeping them early releases the pq/pk psum slots for v units
            for s in range(2):
                nc.scalar.add(qTb[:, s, 0:512], pq[s][64:128, :],
                              bq_sb[64:128, s:s + 1])
                nc.scalar.copy(kTb[:, s, 0:512], pk[s][64:128, :])
            # v00/v01 matmuls + copies (gate h0's first OTs)
            for fn in [unit_v(0, 0), unit_v(0, 1)]:
                fn()

            # ---- flattened attention stream: slots (j, h, pp) with a
            # global LOOKP-2 ST lookahead that CROSSES head and chunk
            # boundaries, so the exp pipeline never drains between heads ----
            slots = []
            base = {}
            for j in range(4):
                for h in range(HL):
                    base[(j, h)] = len(slots)
                    slots += [(j, h, pp) for pp in range(2 * (j + 1))]
            nslots = len(slots)
            heads = {}

            def head_ot(j, h):
                if (j, h) not in heads:
                    heads[(j, h)] = ps.tile([128, 512], F32, tag="ot", bufs=2,
                                            name=f"ot{j}{h}")
                return heads[(j, h)]

            def emit_pair_st(j, h, pp):
                qt = qTa if h < 2 else qTb
                kt = kTa if h < 2 else kTb
                b0 = 32 * (h % 2)
                st = ps.tile([128, 1024], F32, tag="st", bufs=2,
                             name=f"st{j}{h}{pp}")
                pt = ptp.tile([128, 1024], BF16, tag="pt",
                              name=f"pt{j}{h}{pp}")
                i0 = 2 * pp
                qs0 = max(512 * j, 128 * i0)
                n0 = 512 * j + 512 - qs0
                off1 = max(256, n0)
                infos = []
                for half in range(2):
                    i = i0 + half
                    qs = max(512 * j, 128 * i)
                    n = 512 * j + 512 - qs
                    o = off1 * half
                    nc.tensor.matmul(
                        st[:, o:o + n],
                        kt[b0:b0 + 32, :, 128 * i:128 * i + 128],
                        qt[b0:b0 + 32, :, qs:qs + n],
                        start=True, stop=True, perf_mode=DR)
                    infos.append((i, qs, n, o))
                nc.scalar.activation(pt[:, 0:off1 + infos[1][2]],
                                     st[:, 0:off1 + infos[1][2]],
                                     AF.Exp, scale=ESCALE)
                if i0 >= 4 * j:  # diagonal pair: causal triangles
                    # zero the above-diagonal exp values via a 0/1 triangle
                    # multiply on Pool (bf16 SBUF op, off the DVE/ACT path)
                    ptv = pt[:, 0:2 * off1].rearrange("p (a c) -> p a c", a=2)
                    nc.gpsimd.tensor_tensor(
                        ptv[:, :, 0:128], ptv[:, :, 0:128], mask2_sb,
                        op=OP.mult)
                return pt, infos

            def emit_ot_pair(j, h, pt, infos):
                ot = head_ot(j, h)
                ni = 4 * (j + 1)
                for i, qs, n, o in infos:
                    nc.tensor.matmul(
                        ot[:, qs - 512 * j:512],
                        vO_sb[i][:, 128 * h:128 * h + 128],
                        pt[:, o:o + n],
                        start=(i == 0), stop=(i == ni - 1))

            def normalize(j, h):
                # ot rows 64:128 hold broadcast denominators
                ot = heads[(j, h)]
                rb = rcp.tile([64, 512], F32, tag=f"rb{h}", name=f"rb{j}{h}")
                ydst = yT_sb[h // 2][64 * (h % 2):64 * (h % 2) + 64,
                                    512 * j:512 * j + 512]
                if j == 3 and h == 3:
                    # quartered: epilogue minis unblock progressively
                    for qt_ in range(4):
                        cs = slice(128 * qt_, 128 * qt_ + 128)
                        nc.vector.reciprocal(rb[:, cs], ot[64:128, cs])
                        nc.vector.tensor_tensor(
                            ydst[:, cs], ot[0:64, cs], rb[:, cs], op=OP.mult)
                else:
                    nc.vector.reciprocal(rb, ot[64:128, :])
                    nc.vector.tensor_tensor(ydst, ot[0:64, :], rb, op=OP.mult)

            def all_fills(j):
                if j == 0:
                    return {
                        0: [unit_v(0, 2), unit_v(0, 3)],   # prologue wave2
                        1: [unit_qk("q", 1, 0), unit_qk("k", 1, 0)],
                        2: [unit_qk("q", 1, 1), unit_qk("k", 1, 1)],
                        3: [],
                    }
                if j == 3:
                    return {
                        0: [unit_v(3, si) for si in range(4)],
                        1: pj_minis(1, 0) + pj_minis(1, 1),
                        2: pj_minis(2, 0) + pj_minis(2, 1),
                        3: pj_minis(3, 0, m2s=(0,)) + pj_minis(3, 1, m2s=(0,)),
                    }
                return {
                    0: [unit_v(j, si) for si in range(4)],
                    1: [unit_qk("q", j + 1, 0), unit_qk("k", j + 1, 0)],
                    2: [unit_qk("q", j + 1, 1), unit_qk("k", j + 1, 1)],
                    3: pj_minis(0, 0) + pj_minis(0, 1) if j == 2 else [],
                }

            pops = {}

            def enter_j(j):
                if j + 2 < 4:
                    dma_x8(j + 2)
                if j + 1 < 4:
                    dma_xbf(j + 1)
                if j == 0:
                    dma_wp()
                npairs = 2 * (j + 1)
                for h, fill in all_fills(j).items():
                    for k, fn in enumerate(fill):
                        pos = (k * npairs) // len(fill)
                        if j == 3 and h >= 2:
                            pos += 2
                        if h == 0 and j > 0:
                            pos = min(pos, 2 * j - 1)
                        pops.setdefault(base[(j, h)] + min(pos, npairs - 1),
                                        []).append(fn)

            LOOKP = 2
            live = {}
            enter_j(0)
            for i in range(min(LOOKP, nslots)):
                live[i] = emit_pair_st(*slots[i])
            for i in range(nslots):
                j, h, pp = slots[i]
                if h == 0 and pp == 0 and j > 0:
                    enter_j(j)
                for fn in pops.pop(i, []):
                    fn()
                if i + LOOKP < nslots:
                    live[i + LOOKP] = emit_pair_st(*slots[i + LOOKP])
                emit_ot_pair(j, h, *live.pop(i))
                if pp == 2 * (j + 1) - 1:
                    normalize(j, h)
            # epilogue: m1-half of the last 512 tokens' projection -> out2,
            # minis cycling over freed st/ot/mm psum tags; copies 5:3 ACT:DVE
            tagcycle = ["st", "ot", "mm", "st", "ot", "mm", "st", "ot"]
            cpcycle = ["a", "a", "a", "a", "a", "v", "v", "v"]
            for sp in range(2):
                for mini in pj_minis(3, sp, m2s=(1,), fine=True,
                                     tags=tagcycle[4 * sp:4 * sp + 4],
                                     cps=cpcycle[4 * sp:4 * sp + 4]):
                    mini()

    _install_legalizer(nc)
    return nc


_NC_CACHE = None


def _get_nc():
    global _NC_CACHE
    if _NC_CACHE is None:
        _NC_CACHE = build_nc()
    return _NC_CACHE


def _bf16(a: np.ndarray) -> np.ndarray:
    return np.ascontiguousarray(np.asarray(a, np.float32)).astype(ml_dtypes.bfloat16)


def _e4(a: np.ndarray) -> np.ndarray:
    return np.ascontiguousarray(np.asarray(a, np.float32)).astype(
        ml_dtypes.float8_e4m3)


def _pack_w8(w_local: np.ndarray) -> np.ndarray:
    """[256, 1024] fp32 (rows = local q-dims) -> fp8 [64, 4096] in
    [p6, (s_out, kc, s_in, pi)] DoubleRow layout, pre-scaled x32."""
    w8 = _e4(np.asarray(w_local, np.float32) * 32.0)
    a = w8.reshape(4, 2, 32, 8, 2, 64)          # [hq, so, r, kc, si, p6]
    return np.ascontiguousarray(
        a.transpose(5, 1, 3, 4, 0, 2).reshape(64, 4096))


def _pack_x8(xTb: np.ndarray) -> np.ndarray:
    """x[b].T [1024, 2048] fp32 -> fp8 [64, 8*2*S] in [p6, (kc, s, t)]."""
    x8 = _e4(xTb)
    a = x8.reshape(8, 2, 64, S)                 # [kc, s, p6, t]
    return np.ascontiguousarray(a.transpose(2, 0, 1, 3).reshape(64, 8 * 2 * S))


def _pack_bq(bq_local: np.ndarray) -> np.ndarray:
    """[256] -> f32 [128, 2] = 32*bq in [pi, s_out] order."""
    b = (np.asarray(bq_local, np.float32) * 32.0).reshape(4, 2, 32)
    return np.ascontiguousarray(b.transpose(0, 2, 1).reshape(128, 2))


def make_in_maps(x, Wq, bq, Wk, Wv, Wp):
    x = np.asarray(x, np.float32)
    xT = [x[b].T for b in range(2)]
    xTb = [_bf16(t) for t in xT]
    x8b = [_pack_x8(t) for t in xT]
    tri = np.where(np.arange(128)[None, :] >= np.arange(128)[:, None],
                   np.float32(1.0), np.float32(0.0))
    tri2 = _bf16(np.concatenate([tri, tri], axis=1))  # [128, 256] duplicated
    in_maps = []
    for c in range(8):
        b, g = c // 4, c % 4
        sl = slice(DL * g, DL * g + DL)
        in_maps.append({
            "xT": xTb[b],
            "x8": x8b[b],
            "wq8": _pack_w8(np.asarray(Wq)[sl, :]),
            "wk8": _pack_w8(np.asarray(Wk)[sl, :]),
            "wv": _bf16(np.asarray(Wv)[sl, :].T),
            "wp": _bf16(np.asarray(Wp)[:, sl].T),
            "bq": _pack_bq(np.asarray(bq, np.float32)[sl]),
            "mask": tri2,
        })
    return in_maps


def kernel(x, Wq, bq, Wk, bk, Wv, bv, Wp, bp, _run_kwargs=None):
    nc = _get_nc()
    in_maps = make_in_maps(x, Wq, bq, Wk, Wv, Wp)
    res = run_bass_kernel_spmd(nc, in_maps, list(range(8)), **(_run_kwargs or {}))
    corr = (np.asarray(bv, np.float32) @ np.asarray(Wp, np.float32).T
            + np.asarray(bp, np.float32))
    out = np.zeros((2, S, D), np.float32)
    for c in range(8):
        out[c // 4] += np.asarray(res.results[c]["out"]).astype(np.float32)
        out[c // 4, 1536:2048] += np.asarray(
            res.results[c]["out2"]).astype(np.float32)
    out += corr[None, None, :]
    kernel.last_results = res
    return out


# revision 56
# speedup vs baseline: 1.2181x; 1.0194x over previous
"""Causal self-attention kernel for Trainium2, 8 NeuronCores (fp8 v2).

Problem: B=2, S=2048, D=1024, H=16 heads, Hd=64. fp32 in/out.
  q/k/v = x @ W{q,k,v}.T + b;  att = softmax(causal(q k^T / 8));  y = att v
  out = y @ Wp.T + bp

Sharding (batch x head-group): core c -> batch b=c//4, head-group g=c%4
(4 heads = 256 of 1024 dims). Each core computes its QKV slice on its
batch, causal attention for its 4 heads, and a partial output projection
out_c = y_c @ Wp[:, g-slice].T (row-parallel TP). Host unshard:
out[b] = sum_g out_partial[4b+g] + (bv @ Wp.T + bp).

fp8 acceleration (validated vs the 2e-2 gate; numpy study ~1.4e-2):
  - q/k PRODUCTION runs in fp8e4m3 DoubleRow (x8 @ 32*Wq8): both operands
    fp8, 2 contraction rows/PE-cycle => half the bf16 matmul time. Weights
    scaled x32 into e4m3 normal range (x32 is exponent-exact in fp8/psum);
    bias 32*bq folded in the psum->sbuf copy; exp scale becomes 0.125/1024.
  - SCORES (q k^T) also run DoubleRow: qT/kT live as fp8 [64, 2, S] tiles
    (heads 0-1 / 2-3 per tile, head-dim halves side by side in the free
    dim). The psum->sbuf q/k copies write halves straight (psum[0:64]) and
    partition-REMAPPED (psum[64:128] -> partitions 0-63; engines accept
    differing in/out base partitions - validated on HW).
  - v/OT/projection stay bf16: quantizing v, P, or y to fp8 measured over
    the 2e-2 budget (2.5-3e-2), so OT cost is unchanged.
PE row budget: QKV 65536 + ST 34816 + OT 69632 + proj 32768 = 202752 rows
(~84.5us at 2.4GHz) vs 270336 bf16 rows (~112.6us) in the baseline.

Exact algebraic folds (softmax-invariant / row-sum-1):
  - bk dropped: per-query-row constant -> softmax invariant.
  - bv folded to host: softmax rows sum to 1 -> bv @ Wp.T added on host.

On-device layout: transposed "S.T layout" ([k-part, q-free]) so causal
softmax normalization is per-column, P.T feeds P@V directly as the moving
operand, and V carries 64 ones-columns per head so the matmul emits
broadcast softmax denominators for free. Attention ST blocks run in PAIRS
sharing a 2-bank PSUM tile so each exp covers 2 key-blocks (halves ACT
instruction overhead, ~185ns each).

Scheduling (tuned against TimelineSim engine-occupancy traces; ACT is
the pacing engine for ~90% of the run, so the exp stream must never
drain):
  - The attention loop is FLATTENED into one stream of (j, h, pair)
    slots with a global LOOKP-2 ST lookahead that crosses head and chunk
    boundaries; filler pops BEFORE the lookahead ST so the in-order PE
    never idles behind an ST that waits on exp(i) freeing its psum buf.
  - Causal masking is a 0/1 triangle MULTIPLY on pt AFTER the exp, on
    the otherwise-idle Pool engine (bf16 SBUF op) - the masked region's
    exps are garbage-but-finite and zeroed before the OT reads them.
    This keeps ~12us of mask adds off the DVE/ACT critical path.
  - QKV/projection work drips between slots as PE filler at each
    chunk's exp-slack rate. Chunk j+1's q/k units MUST finish inside
    chunk j (every head needs both head-dim slots before its first ST).
  - Engine split: prologue copies on ACT (idle until the first exp at
    ~10us), all later copies/normalize on DVE; exps own ACT; masks +
    ones-memsets on Pool.
  - Startup: wq8-s0 + x8 (via the Pool SWDGE queue, parallel to the
    ~650ns/DMA HWDGE serialization) feed the first psum groups; the
    head-0/1 "a" copies run first so the first ST unblocks after 4
    copies; x bf16 arrives token-split so v00/v01 meet the first
    diagonal OT. All x8 chunks prefetch 1-2 chunks ahead.
  - j3 hosts proj(1..3) minis since late chunks have the largest
    exp-vs-PE deficit; the last head's normalize is quartered so the
    epilogue's m1-half minis (out2) unblock progressively.
"""
import json
import sys

sys.path.insert(0, "/opt/trn_rl_repo")

import ml_dtypes
import numpy as np

import concourse.bass as bass
import concourse.mybir as mybir
import concourse.tile as tile
from concourse.bass_utils import run_bass_kernel_spmd

F32 = mybir.dt.float32
BF16 = mybir.dt.bfloat16
E4 = mybir.dt.float8e4
AF = mybir.ActivationFunctionType
OP = mybir.AluOpType
DR = mybir.MatmulPerfMode.DoubleRow

S = 2048          # tokens per batch (= per core)
D = 1024          # model dim
HL = 4            # heads per core
HD = 64           # head dim
DL = HL * HD      # local dims per core (256)
MASKVAL = -1e30
ESCALE = 0.125 / 1024.0   # exp scale: 1/sqrt(Hd) / (32*32 weight prescale)


# ---------------------------------------------------------------------------
# Wait-legalization: the walrus backend enforces <=1 sem-wait per instruction
# (<=2 for EventSemaphore); Tile's wait-assignment can attach more. Spill
# extras onto EventSemaphore instructions inserted before the offender.
def _legalize_waits_json(bir_bytes: bytes) -> bytes:
    j = json.loads(bir_bytes)
    for fn in j["functions"]:
        for bb in fn["blocks"]:
            out = []
            for inst in bb["instructions"]:
                si = inst.get("sync_info") or {}
                ws = si.get("on_wait") or []
                cap = 2 if inst.get("opcode") == "EventSemaphore" else 1
                if len(ws) > cap:
                    extras, keep = ws[:-cap], ws[-cap:]
                    k = 0
                    while extras:
                        chunk, extras = extras[:2], extras[2:]
                        out.append({
                            "debug": inst.get("debug", 0),
                            "engine": inst["engine"],
                            "ins": [],
                            "name": f"{inst['name']}_wfix{k}",
                            "opcode": "EventSemaphore",
                            "outs": [],
                            "sync_info": {"on_update": [], "on_wait": chunk},
                        })
                        k += 1
                    si["on_wait"] = keep
                out.append(inst)
            bb["instructions"] = out
    return json.dumps(j).encode()


def _install_legalizer(nc):
    orig = nc.to_json_bytes
    nc.to_json_bytes = lambda: _legalize_waits_json(orig())


def build_nc() -> bass.Bass:
    nc = bass.Bass(trn_type="TRN2", num_devices=8)

    xT = nc.dram_tensor("xT", [D, S], BF16, kind="ExternalInput")      # x[b].T
    # fp8 x for the q/k path, 64-partition layout: [p6, (kc, s, t)] with
    # model-dim d = 128*kc + 64*s + p6 (DoubleRow pair = (p6, s)). All 8
    # kc chunks at base partition 0: the PE rejects accumulation groups
    # whose operands switch base partitions mid-group.
    x8 = nc.dram_tensor("x8", [64, 8 * 2 * S], E4, kind="ExternalInput")
    # fp8 scaled weights: [p6, (s_out, kc, s_in, pi)]; s_out-major so the
    # startup stream fetches one out-slot half with 2KB descriptors
    wq8 = nc.dram_tensor("wq8", [64, 4096], E4, kind="ExternalInput")
    wk8 = nc.dram_tensor("wk8", [64, 4096], E4, kind="ExternalInput")
    wv = nc.dram_tensor("wv", [D, DL], BF16, kind="ExternalInput")     # Wv_g.T
    wp = nc.dram_tensor("wp", [DL, D], BF16, kind="ExternalInput")     # Wp[:,sl].T
    bq = nc.dram_tensor("bq", [128, 2], F32, kind="ExternalInput")     # 32*bq perm
    # 0/1 lower-triangle (duplicated x2): multiplied into pt AFTER the exp
    # on the Pool engine (bf16 SBUF-only op), keeping the causal mask off
    # the DVE/ACT critical path entirely
    mask = nc.dram_tensor("mask", [128, 256], BF16, kind="ExternalInput")
    out = nc.dram_tensor("out", [S, D], BF16, kind="ExternalOutput")
    # m1-half partial projection of the last 512 tokens; host adds it to
    # out[1536:2048] (which holds only the m0 half).
    out2 = nc.dram_tensor("out2", [512, D], BF16, kind="ExternalOutput")

    with tile.TileContext(nc) as tc:
        with tc.tile_pool(name="const", bufs=1) as const, \
             tc.tile_pool(name="acts", bufs=1) as acts, \
             tc.tile_pool(name="xin", bufs=2) as xpool, \
             tc.tile_pool(name="pt", bufs=5) as ptp, \
             tc.tile_pool(name="rc", bufs=1) as rcp, \
             tc.tile_pool(name="outp", bufs=4) as outp, \
             tc.tile_pool(name="ps", bufs=1, space="PSUM") as ps:
            wq8_sb = const.tile([64, 2, 8, 2, 128], E4)
            wk8_sb = const.tile([64, 2, 8, 2, 128], E4)
            wv_sb = const.tile([128, 8, DL], BF16)
            wp_sb = const.tile([128, 2, D], BF16)
            bq_sb = const.tile([128, 2], F32)
            mask2_sb = const.tile([128, 2, 128], BF16)

            # q/k in fp8 "DoubleRow ST" layout: tile a = heads 0,1 / b =
            # heads 2,3; head h at partitions 32*(h%2)..+32, head-dim
            # d = 32*slot + partition-offset
            qTa = acts.tile([64, 2, S], E4, name="qTa")
            qTb = acts.tile([64, 2, S], E4, name="qTb")
            kTa = acts.tile([64, 2, S], E4, name="kTa")
            kTb = acts.tile([64, 2, S], E4, name="kTb")
            yT_sb = [acts.tile([128, S], BF16, name=f"yT{m}") for m in range(2)]
            # v with interleaved ones-columns: head h at cols [128h,128h+64) = v,
            # [128h+64,128h+128) = 1.0 -> P@V emits broadcast denominators
            vO_sb = [acts.tile([128, 4 * 128], BF16, name=f"vO{i}") for i in range(16)]

            x3 = xT[:].rearrange("(kc p) t -> p kc t", p=128)
            x83 = x8[:].rearrange("p (kc s t) -> p kc s t", kc=8, s=2)
            wq84 = wq8[:].rearrange("p (so kc si x) -> p so kc si x",
                                    so=2, kc=8, si=2)
            wk84 = wk8[:].rearrange("p (so kc si x) -> p so kc si x",
                                    so=2, kc=8, si=2)
            wv3 = wv[:].rearrange("(kc p) m -> p kc m", p=128)

            xsb_tiles = {}
            x8sb_tiles = {}

            def dma_x8(t4):
                # prefetched TWO chunks ahead: chunk t4's q/k fill units
                # drip during chunk t4-1 and block the in-order PE if their
                # x8 hasn't landed
                x8sb = xpool.tile([64, 8, 2, 512], E4, tag="x8", bufs=4,
                                  name=f"x8{t4}")
                x8sb_tiles[t4] = x8sb
                for half in range(2):
                    nc.sync.dma_start(
                        out=x8sb[:, 4 * half:4 * half + 4],
                        in_=x83[:, 4 * half:4 * half + 4, :,
                                512 * t4:512 * t4 + 512])

            def dma_xbf(t4):
                xsb = xpool.tile([128, 8, 512], BF16, tag="x", name=f"x{t4}")
                xsb_tiles[t4] = xsb
                for half in range(2):
                    nc.sync.dma_start(
                        out=xsb[:, 4 * half:4 * half + 4, :],
                        in_=x3[:, 4 * half:4 * half + 4, 512 * t4:512 * t4 + 512])

            # --- startup stream in prologue-consumption order ---
            x8sb0 = xpool.tile([64, 8, 2, 512], E4, tag="x8", bufs=4,
                               name="x80")
            x8sb_tiles[0] = x8sb0
            xsb0 = xpool.tile([128, 8, 512], BF16, tag="x", name="x0")
            xsb_tiles[0] = xsb0

            def dma_x0tok(tp, n):
                # token-split pieces: v00/v01 (tokens 0-255) unblock early
                sl = slice(128 * tp, 128 * tp + n)
                nc.sync.dma_start(out=xsb0[:, :, sl], in_=x3[:, :, sl])

            # x8 pieces go through the Pool SWDGE queue: desc-gen on the
            # idle Pool engine runs PARALLEL to the HWDGE's ~650ns/DMA
            # serialization, so the weight stream isn't pushed back
            nc.gpsimd.dma_start(out=x8sb0[:, 0:4], in_=x83[:, 0:4, :, 0:512])
            nc.gpsimd.dma_start(out=x8sb0[:, 4:8], in_=x83[:, 4:8, :, 0:512])
            nc.sync.dma_start(out=wq8_sb[:, 0], in_=wq84[:, 0])
            nc.sync.dma_start(out=bq_sb, in_=bq[:])
            nc.sync.dma_start(out=wk8_sb[:, 0], in_=wk84[:, 0])
            nc.sync.dma_start(out=wq8_sb[:, 1], in_=wq84[:, 1])
            nc.sync.dma_start(out=wk8_sb[:, 1], in_=wk84[:, 1])
            nc.sync.dma_start(out=wv_sb, in_=wv3)
            dma_x0tok(0, 256)
            nc.sync.dma_start(
                out=mask2_sb,
                in_=mask[:].rearrange("p (a c) -> p a c", a=2))
            dma_x0tok(2, 256)
            dma_x8(1)

            def dma_wp():
                # deferred: wp first consumed by proj(0) minis at j2-h3
                for m in range(2):
                    nc.sync.dma_start(
                        out=wp_sb[:, m, :],
                        in_=wp[:].rearrange("(m p) n -> p m n", p=128)[:, m, :])
            for i in range(16):
                v3 = vO_sb[i].rearrange("p (h c) -> p h c", h=HL)
                nc.gpsimd.memset(v3[:, :, HD:128], 1.0)

            # --- keyed work units (one PE psum-group each) ---
            def unit_qk(which, t4, s):
                wsb = wq8_sb if which == "q" else wk8_sb
                da, db = (qTa, qTb) if which == "q" else (kTa, kTb)

                def go():
                    ts512 = slice(512 * t4, 512 * t4 + 512)
                    x8sb = x8sb_tiles[t4]
                    p = ps.tile([128, 512], F32, tag="mm", bufs=2,
                                name=f"{which}{t4}{s}")
                    for kc in range(8):
                        nc.tensor.matmul(
                            p[:, :], wsb[:, s, kc, :, :], x8sb[:, kc, :, :],
                            start=(kc == 0), stop=(kc == 7), perf_mode=DR)
                    # psum[0:64] -> heads 0,1 straight; [64:128] -> heads 2,3
                    # partition-remapped into the b tile
                    if which == "q":
                        nc.vector.tensor_scalar_add(
                            da[:, s, ts512], p[0:64, :], bq_sb[0:64, s:s + 1])
                        nc.vector.tensor_scalar_add(
                            db[:, s, ts512], p[64:128, :],
                            bq_sb[64:128, s:s + 1])
                    else:
                        nc.vector.tensor_copy(da[:, s, ts512], p[0:64, :])
                        nc.vector.tensor_copy(db[:, s, ts512], p[64:128, :])
                return go

            def unit_v(t4, si, tag="mm", eng="v"):
                def go():
                    xsb = xsb_tiles[t4]
                    tl = slice(128 * si, 128 * si + 128)
                    shape = [128, 1024] if tag == "st" else [128, DL]
                    p = ps.tile(shape, F32, tag=tag, bufs=2, name=f"v{t4}{si}")
                    for kc in range(8):
                        nc.tensor.matmul(p[:, 0:DL], xsb[:, kc, tl],
                                         wv_sb[:, kc, :],
                                         start=(kc == 0), stop=(kc == 7))
                    v3 = vO_sb[4 * t4 + si].rearrange("p (h c) -> p h c", h=HL)
                    cp = (nc.scalar.copy if eng == "a"
                          else nc.vector.tensor_copy)
                    cp(v3[:, :, 0:HD],
                       p[:, 0:DL].rearrange("p (h c) -> p h c", h=HL))
                return go

            def pj_minis(jj, sp, m2s=(0, 1), fine=False, tags=None, cps=None):
                """Projection of the 256 tokens at 512*jj+256*sp over the
                contraction halves in m2s, as FOUR independent mini-units
                (one psum group each) dripped between attention pairs."""
                t0 = 512 * jj + 256 * sp
                dram = out2 if m2s == (1,) else out
                r0 = t0 - 1536 if m2s == (1,) else t0
                state = {}

                def mini(sub, ncol, tag, cp):
                    def go():
                        if "osb" not in state:
                            state["osb"] = outp.tile(
                                [128, 2, D], BF16, tag="o",
                                name=f"o{jj}{sp}{m2s[0]}{len(m2s)}")
                        osb = state["osb"]
                        shape = [128, 1024] if tag == "st" else [128, 512]
                        pj = ps.tile(shape, F32, tag=tag, bufs=2,
                                     name=f"pj{jj}{sp}{sub}{ncol}{m2s[0]}")
                        for ii, m2 in enumerate(m2s):
                            nc.tensor.matmul(
                                pj[:, 0:512],
                                yT_sb[m2][:, t0 + 128 * sub:t0 + 128 * sub + 128],
                                wp_sb[:, m2, 512 * ncol:512 * ncol + 512],
                                start=(ii == 0), stop=(ii == len(m2s) - 1))
                        dst = osb[:, sub, 512 * ncol:512 * ncol + 512]
                        if cp == "a":
                            nc.scalar.copy(dst, pj[:, 0:512])
                        else:
                            nc.vector.tensor_copy(dst, pj[:, 0:512])
                        if fine and sp == 1 and sub == 1:
                            q = nc.sync if ncol == 1 else nc.scalar
                            q.dma_start(
                                out=dram[r0 + 128:r0 + 256,
                                         512 * ncol:512 * ncol + 512],
                                in_=dst)
                        elif fine and ncol == 1:
                            nc.sync.dma_start(
                                out=dram[r0 + 128 * sub:r0 + 128 * sub + 128, :],
                                in_=osb[:, sub, :])
                        elif not fine and sub == 1 and ncol == 1:
                            nc.sync.dma_start(
                                out=dram[r0:r0 + 256, :].rearrange(
                                    "(a p) n -> p a n", p=128),
                                in_=osb[:])
                    return go

                tags = tags or ["mm"] * 4
                cps = cps or [("a" if (fine and ncol == 1) else "v")
                              for _ in range(2) for ncol in range(2)]
                return [mini(sub, ncol, tags[2 * sub + ncol],
                             cps[2 * sub + ncol])
                        for sub in range(2) for ncol in range(2)]

            # --- prologue: q00 s0/s1 (mm), k00 s0/s1 (ot), v00/v01
            # (st-borrow), each group emitted WHOLE in DMA-arrival order so
            # completions stagger; the head-0/1 "a" copies go first so the
            # first ST unblocks after 4 copies (b copies feed h2, v feeds
            # the first diagonal OT - both later deadlines)
            # psum borrows: q00 -> mm, k00 -> ot, and v00/v01 ALSO -> mm
            # (NOT st: the first attention pair's STs must find fresh st
            # slots, not wait on the v-copy chain behind the big x0/wv DMAs)
            pq = [ps.tile([128, 512], F32, tag="mm", bufs=2, name=f"pq00{s}")
                  for s in range(2)]
            pk = [ps.tile([128, 512], F32, tag="ot", bufs=2, name=f"pk00{s}")
                  for s in range(2)]

            def mm_qk(p, wsb, s):
                for kc in range(8):
                    nc.tensor.matmul(
                        p[:, :], wsb[:, s, kc, :, :], x8sb0[:, kc, :, :],
                        start=(kc == 0), stop=(kc == 7), perf_mode=DR)

            mm_qk(pq[0], wq8_sb, 0)
            mm_qk(pk[0], wk8_sb, 0)
            nc.vector.tensor_scalar_add(qTa[:, 0, 0:512], pq[0][0:64, :],
                                        bq_sb[0:64, 0:1])
            nc.vector.tensor_copy(kTa[:, 0, 0:512], pk[0][0:64, :])
            mm_qk(pq[1], wq8_sb, 1)
            mm_qk(pk[1], wk8_sb, 1)
            nc.vector.tensor_scalar_add(qTa[:, 1, 0:512], pq[1][0:64, :],
                                        bq_sb[0:64, 1:2])
            nc.vector.tensor_copy(kTa[:, 1, 0:512], pk[1][0:64, :])
            # b copies (heads 2,3) on ACT: idle until the first exp, and
            # keeping them early releases the pq/pk psum slots for v units
            for s in range(2):
                nc.scalar.add(qTb[:, s, 0:512], pq[s][64:128, :],
                              bq_sb[64:128, s:s + 1])
                nc.scalar.copy(kTb[:, s, 0:512], pk[s][64:128, :])
            # v00/v01 matmuls + copies (gate h0's first OTs); copies on ACT
            # (idle until the first exp) to relieve the j0-era DVE queue
            for fn in [unit_v(0, 0, eng="a"), unit_v(0, 1, eng="a")]:
                fn()

            # ---- flattened attention stream: slots (j, h, pp) with a
            # global LOOKP-2 ST lookahead that CROSSES head and chunk
            # boundaries, so the exp pipeline never drains between heads ----
            slots = []
            base = {}
            for j in range(4):
                for h in range(HL):
                    base[(j, h)] = len(slots)
                    slots += [(j, h, pp) for pp in range(2 * (j + 1))]
            nslots = len(slots)
            heads = {}

            def head_ot(j, h):
                if (j, h) not in heads:
                    heads[(j, h)] = ps.tile([128, 512], F32, tag="ot", bufs=2,
                                            name=f"ot{j}{h}")
                return heads[(j, h)]

            def emit_pair_st(j, h, pp):
                qt = qTa if h < 2 else qTb
                kt = kTa if h < 2 else kTb
                b0 = 32 * (h % 2)
                st = ps.tile([128, 1024], F32, tag="st", bufs=2,
                             name=f"st{j}{h}{pp}")
                pt = ptp.tile([128, 1024], BF16, tag="pt",
                              name=f"pt{j}{h}{pp}")
                i0 = 2 * pp
                qs0 = max(512 * j, 128 * i0)
                n0 = 512 * j + 512 - qs0
                off1 = max(256, n0)
                infos = []
                for half in range(2):
                    i = i0 + half
                    qs = max(512 * j, 128 * i)
                    n = 512 * j + 512 - qs
                    o = off1 * half
                    nc.tensor.matmul(
                        st[:, o:o + n],
                        kt[b0:b0 + 32, :, 128 * i:128 * i + 128],
                        qt[b0:b0 + 32, :, qs:qs + n],
                        start=True, stop=True, perf_mode=DR)
                    infos.append((i, qs, n, o))
                nc.scalar.activation(pt[:, 0:off1 + infos[1][2]],
                                     st[:, 0:off1 + infos[1][2]],
                                     AF.Exp, scale=ESCALE)
                if i0 >= 4 * j:  # diagonal pair: causal triangles
                    # zero the above-diagonal exp values via a 0/1 triangle
                    # multiply on Pool (bf16 SBUF op, off the DVE/ACT path)
                    ptv = pt[:, 0:2 * off1].rearrange("p (a c) -> p a c", a=2)
                    nc.gpsimd.tensor_tensor(
                        ptv[:, :, 0:128], ptv[:, :, 0:128], mask2_sb,
                        op=OP.mult)
                return pt, infos

            def emit_ot_pair(j, h, pt, infos):
                ot = head_ot(j, h)
                ni = 4 * (j + 1)
                for i, qs, n, o in infos:
                    nc.tensor.matmul(
                        ot[:, qs - 512 * j:512],
                        vO_sb[i][:, 128 * h:128 * h + 128],
                        pt[:, o:o + n],
                        start=(i == 0), stop=(i == ni - 1))

            def normalize(j, h):
                # ot rows 64:128 hold broadcast denominators
                ot = heads[(j, h)]
                rb = rcp.tile([64, 512], F32, tag=f"rb{h}", name=f"rb{j}{h}")
                ydst = yT_sb[h // 2][64 * (h % 2):64 * (h % 2) + 64,
                                    512 * j:512 * j + 512]
                if j == 3 and h == 3:
                    # quartered: epilogue minis unblock progressively
                    for qt_ in range(4):
                        cs = slice(128 * qt_, 128 * qt_ + 128)
                        nc.vector.reciprocal(rb[:, cs], ot[64:128, cs])
                        nc.vector.tensor_tensor(
                            ydst[:, cs], ot[0:64, cs], rb[:, cs], op=OP.mult)
                else:
                    nc.vector.reciprocal(rb, ot[64:128, :])
                    nc.vector.tensor_tensor(ydst, ot[0:64, :], rb, op=OP.mult)

            def all_fills(j):
                if j == 0:
                    return {
                        0: [unit_v(0, 2), unit_v(0, 3)],   # prologue wave2
                        1: [unit_qk("q", 1, 0), unit_qk("k", 1, 0)],
                        2: [unit_qk("q", 1, 1), unit_qk("k", 1, 1)],
                        3: [],
                    }
                if j == 3:
                    return {
                        0: [unit_v(3, si) for si in range(4)],
                        1: pj_minis(1, 0) + pj_minis(1, 1),
                        2: pj_minis(2, 0) + pj_minis(2, 1),
                        3: pj_minis(3, 0, m2s=(0,)) + pj_minis(3, 1, m2s=(0,)),
                    }
                return {
                    0: [unit_v(j, si) for si in range(4)],
                    1: [unit_qk("q", j + 1, 0), unit_qk("k", j + 1, 0)],
                    2: [unit_qk("q", j + 1, 1), unit_qk("k", j + 1, 1)],
                    3: pj_minis(0, 0) + pj_minis(0, 1) if j == 2 else [],
                }

            pops = {}

            def enter_j(j):
                if j + 2 < 4:
                    dma_x8(j + 2)
                if j + 1 < 4:
                    dma_xbf(j + 1)
                if j == 0:
                    dma_wp()
                npairs = 2 * (j + 1)
                for h, fill in all_fills(j).items():
                    for k, fn in enumerate(fill):
                        pos = (k * npairs) // len(fill)
                        if j == 3 and h >= 2:
                            pos += 2
                        if h == 0 and j > 0:
                            pos = min(pos, 2 * j - 1)
                        pops.setdefault(base[(j, h)] + min(pos, npairs - 1),
                                        []).append(fn)

            LOOKP = 2
            live = {}
            enter_j(0)
            for i in range(min(LOOKP, nslots)):
                live[i] = emit_pair_st(*slots[i])
            for i in range(nslots):
                j, h, pp = slots[i]
                if h == 0 and pp == 0 and j > 0:
                    enter_j(j)
                for fn in pops.pop(i, []):
                    fn()
                if i + LOOKP < nslots:
                    live[i + LOOKP] = emit_pair_st(*slots[i + LOOKP])
                emit_ot_pair(j, h, *live.pop(i))
                if pp == 2 * (j + 1) - 1:
                    normalize(j, h)
            # epilogue: m1-half of the last 512 tokens' projection -> out2,
            # minis cycling over freed st/ot/mm psum tags; copies 5:3 ACT:DVE
            tagcycle = ["st", "ot", "mm", "st", "ot", "mm", "st", "ot"]
            cpcycle = ["a", "a", "a", "a", "a", "v", "v", "v"]
            for sp in range(2):
                for mini in pj_minis(3, sp, m2s=(1,), fine=True,
                                     tags=tagcycle[4 * sp:4 * sp + 4],
                                     cps=cpcycle[4 * sp:4 * sp + 4]):
                    mini()

    _install_legalizer(nc)
    return nc


_NC_CACHE = None


def _get_nc():
    global _NC_CACHE
    if _NC_CACHE is None:
        _NC_CACHE = build_nc()
    return _NC_CACHE


def _bf16(a: np.ndarray) -> np.ndarray:
    return np.ascontiguousarray(np.asarray(a, np.float32)).astype(ml_dtypes.bfloat16)


def _e4(a: np.ndarray) -> np.ndarray:
    return np.ascontiguousarray(np.asarray(a, np.float32)).astype(
        ml_dtypes.float8_e4m3)


def _pack_w8(w_local: np.ndarray) -> np.ndarray:
    """[256, 1024] fp32 (rows = local q-dims) -> fp8 [64, 4096] in
    [p6, (s_out, kc, s_in, pi)] DoubleRow layout, pre-scaled x32."""
    w8 = _e4(np.asarray(w_local, np.float32) * 32.0)
    a = w8.reshape(4, 2, 32, 8, 2, 64)          # [hq, so, r, kc, si, p6]
    return np.ascontiguousarray(
        a.transpose(5, 1, 3, 4, 0, 2).reshape(64, 4096))


def _pack_x8(xTb: np.ndarray) -> np.ndarray:
    """x[b].T [1024, 2048] fp32 -> fp8 [64, 8*2*S] in [p6, (kc, s, t)]."""
    x8 = _e4(xTb)
    a = x8.reshape(8, 2, 64, S)                 # [kc, s, p6, t]
    return np.ascontiguousarray(a.transpose(2, 0, 1, 3).reshape(64, 8 * 2 * S))


def _pack_bq(bq_local: np.ndarray) -> np.ndarray:
    """[256] -> f32 [128, 2] = 32*bq in [pi, s_out] order."""
    b = (np.asarray(bq_local, np.float32) * 32.0).reshape(4, 2, 32)
    return np.ascontiguousarray(b.transpose(0, 2, 1).reshape(128, 2))


def make_in_maps(x, Wq, bq, Wk, Wv, Wp):
    x = np.asarray(x, np.float32)
    xT = [x[b].T for b in range(2)]
    xTb = [_bf16(t) for t in xT]
    x8b = [_pack_x8(t) for t in xT]
    tri = np.where(np.arange(128)[None, :] >= np.arange(128)[:, None],
                   np.float32(1.0), np.float32(0.0))
    tri2 = _bf16(np.concatenate([tri, tri], axis=1))  # [128, 256] duplicated
    in_maps = []
    for c in range(8):
        b, g = c // 4, c % 4
        sl = slice(DL * g, DL * g + DL)
        in_maps.append({
            "xT": xTb[b],
            "x8": x8b[b],
            "wq8": _pack_w8(np.asarray(Wq)[sl, :]),
            "wk8": _pack_w8(np.asarray(Wk)[sl, :]),
            "wv": _bf16(np.asarray(Wv)[sl, :].T),
            "wp": _bf16(np.asarray(Wp)[:, sl].T),
            "bq": _pack_bq(np.asarray(bq, np.float32)[sl]),
            "mask": tri2,
        })
    return in_maps


def kernel(x, Wq, bq, Wk, bk, Wv, bv, Wp, bp, _run_kwargs=None):
    nc = _get_nc()
    in_maps = make_in_maps(x, Wq, bq, Wk, Wv, Wp)
    res = run_bass_kernel_spmd(nc, in_maps, list(range(8)), **(_run_kwargs or {}))
    corr = (np.asarray(bv, np.float32) @ np.asarray(Wp, np.float32).T
            + np.asarray(bp, np.float32))
    out = np.zeros((2, S, D), np.float32)
    for c in range(8):
        out[c // 4] += np.asarray(res.results[c]["out"]).astype(np.float32)
        out[c // 4, 1536:2048] += np.asarray(
            res.results[c]["out2"]).astype(np.float32)
    out += corr[None, None, :]
    kernel.last_results = res
    return out
